# revision 56
# baseline (speedup 1.0000x reference)
"""CANet (channel-attention net) Trainium2 kernel.

8-core data parallel: 2 samples per core. Full inputs in, full output out.

Per sample the reference computes, for x in {c4,c3,c2} ([C,N] with N=H*W):
    E = x @ x.T                      (symmetric)
    att = softmax(rowmax(E) - E)     == softmax(-E) row-wise
    out = gamma * (att @ x) + x
followed by convs down to [512,16,16], cam_k = conv_k(out_k) + down4,
feat = concat([cam2, cam3, cam4, c4]), pooled = mean(feat, spatial),
logits = pooled @ fc_w.T + fc_b.

Key identity: with global scalar bias M, att = diag(1/z) K with
K = exp(M - E) SYMMETRIC, z = K @ 1, so K's row-major tiles serve directly
as matmul lhsT tiles for att @ x (no transposes); 1/z and gamma fold into
the epilogue. M = (gmax + gmin)/2 + 5 over row-mins keeps exp in range
(CAM2's spread is too wide, so it uses exact per-row bias + PE transposes).

Performance structure:
  - ONE global PSUM pool of 8 bank tiles ([P,512] f32 = one 2KB bank each)
    rotated manually through every phase -- no per-stage psum pools, so no
    conservative pool-boundary sync barriers; all ordering is data deps.
  - Energy tiles accumulate in PSUM 512-col chunks; PSUM->SBUF copy is
    fused with the row-min on DVE (tensor_tensor_reduce) for even chunks,
    Act copy + stride-4 subsampled DVE min for odd chunks.
  - The stabilizer bias needs only global extrema of row-mins: running
    [P,2] extrema on DVE + one gpsimd partition_all_reduce at the end.
  - att matmuls run j-outer with all output tiles PSUM-resident: row-tile
    j of K is exponentiated on Act right before PE consumes it, so the exp
    overlaps the att matmuls instead of serializing before them.
  - Conv weight pools open before the attention stage that precedes them,
    letting weight DMAs prefetch during attention.
"""

import numpy as np

import bass_rust
import concourse.bass as bass
import concourse.bass_isa as bass_isa
import concourse.mybir as mybir
import concourse.tile as tile
from concourse.bass_utils import run_bass_kernel_spmd
from concourse.tile import ScopedClock

F32 = mybir.dt.float32
F32R = mybir.dt.float32r
BF16 = mybir.dt.bfloat16
AX = mybir.AxisListType
OP = mybir.AluOpType
AF = mybir.ActivationFunctionType

P = 128
NCORES = 8
SAMPLES_PER_CORE = 2


# ---------------------------------------------------------------------------
# walrus in this container supports at most ONE sync-wait per instruction;
# split extras across NOPs (tail drain + scheduled instructions).
# ---------------------------------------------------------------------------
_wsplit_counter = [0]


def _fresh_name(base):
    _wsplit_counter[0] += 1
    return f"{base}-wsplit{_wsplit_counter[0]}"


def _patched_drain_and_barrier(self, tick_clock, wait_clock):
    drain_inst = self.nc.sync.drain()
    wait_clock.add_sem_waits(
        drain_inst.ins, ScopedClock({None: tick_clock.global_clock})
    )
    ins = drain_inst.ins
    si = ins.sync_info
    waits = list(si.on_wait) if si is not None else []
    if len(waits) > 1:
        ins.sync_info = bass_rust.SyncInfo(on_wait=waits[:1], on_update=[])
        for w in waits[1:]:
            nop = self.nc.sync.nop(nofuse=True, hint="tail_wait_split")
            nop.ins.sync_info = bass_rust.SyncInfo(on_wait=[w], on_update=[])
    self.nc.all_engine_barrier()
    assert self.sems is not None
    popped = self.nc._tile_sem_poison_stack.pop()
    assert popped is self._sem_poison
    self.nc.clear_and_free_semaphores(list(self.sems.allocated().values()))
    self.nc.all_engine_barrier()


_orig_add_instruction = tile.TileContext._add_instruction


def _split_add_instruction(self, inst):
    si = inst.sync_info
    if si is not None:
        waits = list(si.on_wait)
        if len(waits) > 1:
            for w in waits[:-1]:
                nop = mybir.InstNoOp(name=_fresh_name(inst.name), ins=[], outs=[])
                nop.engine = inst.engine
                nop.sync_info = bass_rust.SyncInfo(on_wait=[w], on_update=[])
                _orig_add_instruction(self, nop)
            inst.sync_info = bass_rust.SyncInfo(
                on_wait=waits[-1:], on_update=list(si.on_update)
            )
    _orig_add_instruction(self, inst)


def _install_tilefix():
    tile.TileContext._drain_and_barrier = _patched_drain_and_barrier
    tile.TileContext._add_instruction = _split_add_instruction


# ---------------------------------------------------------------------------
# kernel program
# ---------------------------------------------------------------------------

def _f32(ap):
    return ap.bitcast(F32)


def _cam_stage(nc, *, name, s, y_tile, x_view, E, PS, gamma_b,
               out_write, small, C, N, att_rhs_chunks, ident,
               ones_row, post_energy=None, ebanks=None, dbg=None):
    """One sample's channel-attention: energy -> bias -> interleaved exp+att.

    PS: the 8 global [P,512] psum bank tiles (manual rotation).
    """
    MT = C // P
    KT = N // P
    NCH = C // 512              # 512-col energy chunks per row-tile
    runx = small.tile([P, 2], F32R, tag=f"runx_{name}", name="runx")
    eb = list(range(8)) if ebanks is None else ebanks

    # --- energy: E[m] = x x^T row-tile, 512-col psum chunks ---
    rot = 0
    for m in range(MT):
        hmin = small.tile([P, NCH], F32, tag=f"hmin_{name}", bufs=2,
                          name="hmin")
        for h in range(NCH):
            ps = PS[eb[rot % len(eb)]]
            rot += 1
            for k in range(KT):
                nc.tensor.matmul(
                    ps[:],
                    y_tile[:, k, m * P:(m + 1) * P],
                    y_tile[:, k, h * 512:(h + 1) * 512],
                    start=(k == 0), stop=(k == KT - 1),
                )
            # psum -> E copy: 2/3 on Act, 1/3 on DVE (balances engine
            # load against the PE cadence); row-min is a stride-4
            # subsampled DVE reduce straight from PSUM -- the stabilizer
            # needs only approximate extrema (midpoint+5 has ~60 e-folds
            # of slack vs the few-unit subsample error)
            if (m * NCH + h) % 3 == 2:
                nc.vector.tensor_copy(
                    E[:, m, h * 512:(h + 1) * 512], ps[:])
            else:
                nc.scalar.activation(
                    E[:, m, h * 512:(h + 1) * 512], ps[:],
                    AF.Identity)
            nc.vector.tensor_reduce(
                hmin[:, h:h + 1], ps[:, 0:512:8] if NCH == 4
                else ps[:, 0:512:4],
                axis=AX.X, op=OP.min)
        if NCH > 1:
            tmin = small.tile([P, 1], F32, tag=f"tmin_{name}", bufs=2,
                              name="tmin")
            nc.vector.tensor_reduce(tmin[:], hmin[:], axis=AX.X, op=OP.min)
        else:
            tmin = hmin
        ntmin = small.tile([P, 1], F32, tag=f"ntmin_{name}", bufs=2,
                           name="ntmin")
        nc.vector.tensor_scalar(ntmin[:], tmin[:], -1.0, None, op0=OP.mult)
        if m == 0:
            nc.vector.tensor_copy(runx[:, 0:1], tmin[:])
            nc.vector.tensor_copy(runx[:, 1:2], ntmin[:])
        else:
            nc.vector.tensor_tensor(runx[:, 0:1], _f32(runx[:, 0:1]),
                                    tmin[:], op=OP.max)
            nc.vector.tensor_tensor(runx[:, 1:2], _f32(runx[:, 1:2]),
                                    ntmin[:], op=OP.max)

    if post_energy is not None:
        post_energy()

    # --- global stabilizer: bias = (gmax + gmin)/2 + 5 over row-mins ---
    # cross-partition via two tiny PE transposes (this walrus cannot
    # compile partition_all_reduce): runx [P,2] -> [2,P] -> per-col max
    # -> [2,4] -> [4,4] -> scalars on partition 0 -> ones-matmul bcast.
    tpa = PS[eb[0]][0:2, 0:128]
    nc.tensor.transpose(tpa.bitcast(F32R), runx[:], ident[:])
    mm = small.tile([2, 4], F32R, tag=f"mm_{name}", name="mm")
    nc.vector.tensor_reduce(mm[:, 0:1], tpa, axis=AX.X, op=OP.max)
    nc.vector.tensor_copy(mm[:, 1:2], _f32(mm[:, 0:1]))
    nc.vector.tensor_copy(mm[:, 2:4], _f32(mm[:, 0:2]))
    tpb = PS[eb[1]][0:4, 0:4]
    nc.tensor.transpose(tpb.bitcast(F32R), mm[:], ident[0:2, 0:4])
    mm2s = small.tile([1, 4], F32, tag=f"mm2s_{name}", name="mm2s")
    nc.vector.tensor_copy(mm2s[:], tpb[0:1, 0:4])
    bias1 = small.tile([1, 1], F32, tag=f"bias1_{name}", name="bias1")
    nc.vector.tensor_tensor(bias1[:], mm2s[:, 0:1], mm2s[:, 1:2],
                            op=OP.subtract)
    nc.vector.tensor_scalar(bias1[:], bias1[:], 0.5, 5.0, op0=OP.mult,
                            op1=OP.add)
    bps = PS[eb[2]][0:P, 0:1]
    nc.tensor.matmul(bps, _f32(ones_row[:]), bias1[:],
                     start=True, stop=True)
    biasP = small.tile([P, 1], F32, tag=f"biasP_{name}", name="biasP")
    nc.vector.tensor_copy(biasP[:], bps)

    # --- att @ x with exp(tile j) interleaved right before first use ---
    z = small.tile([P, MT], F32, tag=f"z_{name}", name="zrow")
    gz = small.tile([P, MT], F32, tag=f"gz_{name}", name="gzrow")
    if len(att_rhs_chunks) == 1:
        c0, cn = att_rhs_chunks[0]
        per = 512 // cn             # m-tiles per psum bank
        views = [PS[i][:].rearrange("p (a b) -> p a b", a=per)
                 for i in range(8)]
        for j in range(MT):
            nc.scalar.activation(
                E[:, j, :], _f32(E[:, j, :]), AF.Exp,
                bias=biasP[:], scale=-1.0, accum_out=z[:, j:j + 1])
            for m in range(MT):
                nc.tensor.matmul(
                    views[m // per][:, m % per, :],
                    E[:, j, m * P:(m + 1) * P], x_view(j)[:, c0:c0 + cn],
                    start=(j == 0), stop=(j == MT - 1))
        rz = small.tile([P, MT], F32, tag=f"rz_{name}", name="rzrow")
        nc.vector.reciprocal(rz[:], z[:])
        nc.vector.tensor_scalar(gz[:], rz[:], gamma_b[:, 0:1], None,
                                op0=OP.mult)
        for m in range(MT):
            out_write(m, c0, cn, views[m // per][:, m % per, :],
                      gz[:, m:m + 1])
    else:
        for h, (c0, cn) in enumerate(att_rhs_chunks):
            for j in range(MT):
                if h == 0:
                    nc.scalar.activation(
                        E[:, j, :], _f32(E[:, j, :]), AF.Exp,
                        bias=biasP[:], scale=-1.0, accum_out=z[:, j:j + 1])
                for m in range(MT):
                    nc.tensor.matmul(
                        PS[m][:], E[:, j, m * P:(m + 1) * P],
                        x_view(j)[:, c0:c0 + cn],
                        start=(j == 0), stop=(j == MT - 1))
            if h == 0:
                rz = small.tile([P, MT], F32, tag=f"rz_{name}", name="rzrow")
                nc.vector.reciprocal(rz[:], z[:])
                nc.vector.tensor_scalar(gz[:], rz[:], gamma_b[:, 0:1],
                                        None, op0=OP.mult)
            for m in range(MT):
                out_write(m, c0, cn, PS[m][:], gz[:, m:m + 1])

    if dbg is not None:
        nc.sync.dma_start(dbg[f"bias_{name}_{s}"][:], bias1[:])
        nc.sync.dma_start(dbg[f"z_{name}_{s}"][:], z[:])
        nc.sync.dma_start(dbg[f"K0_{name}_{s}"][:], _f32(E[:, 0, :]))
    return gz


PHASE_MARKS = []


def _mark(nc, name):
    PHASE_MARKS.append((name, nc.next_id()))


def build_program(debug=False):
    _install_tilefix()
    PHASE_MARKS.clear()
    nc = bass.Bass(name="canet", dynamic_dma_scratch_size=2048)

    S = SAMPLES_PER_CORE
    dbg = None
    if debug:
        dbg = {}
        for nm, mt, c in (("c4", 16, 2048), ("c3", 8, 1024), ("c2", 4, 512)):
            for s in range(S):
                dbg[f"bias_{nm}_{s}"] = nc.dram_tensor(
                    f"dbg_bias_{nm}_{s}", [1, 1], F32, kind="ExternalOutput")
                dbg[f"z_{nm}_{s}"] = nc.dram_tensor(
                    f"dbg_z_{nm}_{s}", [P, mt], F32, kind="ExternalOutput")
                dbg[f"K0_{nm}_{s}"] = nc.dram_tensor(
                    f"dbg_K0_{nm}_{s}", [P, c], F32, kind="ExternalOutput")
        dbg["out4"] = nc.dram_tensor(
            "dbg_out4", [P, S, 16, 256], F32R, kind="ExternalOutput")
        dbg["down4"] = nc.dram_tensor(
            "dbg_down4", [P, S, 4, 256], F32, kind="ExternalOutput")
        dbg["pooled"] = nc.dram_tensor(
            "dbg_pooled", [P, S, 28], F32, kind="ExternalOutput")
        dbg["out3p"] = nc.dram_tensor(
            "dbg_out3p", [P, S, 8, 34, 34], BF16, kind="ExternalOutput")
        dbg["out2p"] = nc.dram_tensor(
            "dbg_out2p", [P, 4, 66, 66], BF16, kind="ExternalOutput")
        dbg["a2p"] = nc.dram_tensor(
            "dbg_a2p", [P, 4, 34, 34], BF16, kind="ExternalOutput")
    x4_d = nc.dram_tensor("x4", [S, 2048, 256], F32R, kind="ExternalInput")
    y4_d = nc.dram_tensor("y4", [S, 256, 2048], F32R, kind="ExternalInput")
    x3_d = nc.dram_tensor("x3", [S, 1024, 1024], F32R, kind="ExternalInput")
    y3_d = nc.dram_tensor("y3", [S, 1024, 1024], F32R, kind="ExternalInput")
    x2_d = nc.dram_tensor("x2", [S, 512, 4096], F32R, kind="ExternalInput")
    y2_d = nc.dram_tensor("y2", [S, 4096, 512], F32R, kind="ExternalInput")
    w4_d = nc.dram_tensor("w4t", [2048, 512], F32R, kind="ExternalInput")
    w3_d = nc.dram_tensor("w3t", [9, 1024, 512], BF16, kind="ExternalInput")
    w2a_d = nc.dram_tensor("w2at", [9, 512, 512], BF16, kind="ExternalInput")
    w2b_d = nc.dram_tensor("w2bt", [9, 512, 512], BF16, kind="ExternalInput")
    b4_d = nc.dram_tensor("b4c", [P, 4], F32, kind="ExternalInput")
    b2b_d = nc.dram_tensor("b2bc", [P, 4], F32, kind="ExternalInput")
    fcw_d = nc.dram_tensor("fcwt", [3584, 2], F32, kind="ExternalInput")
    fcb_d = nc.dram_tensor("fcb", [2], F32, kind="ExternalInput")
    g2_d = nc.dram_tensor("g2", [1], F32, kind="ExternalInput")
    g3_d = nc.dram_tensor("g3", [1], F32, kind="ExternalInput")
    g4_d = nc.dram_tensor("g4", [1], F32, kind="ExternalInput")
    ident_d = nc.dram_tensor("ident", [P, 128], F32R, kind="ExternalInput")
    out_d = nc.dram_tensor("out", [S, 2], F32, kind="ExternalOutput")

    with tile.TileContext(nc) as tc:
        with tc.tile_pool(name="persist", bufs=1) as persist, \
             tc.tile_pool(name="scratch", bufs=2) as scratch, \
             tc.tile_pool(name="gpsum", bufs=1, space="PSUM") as gpsum:

            # global psum register file: 8 bank tiles, rotated manually
            PS = [gpsum.tile([P, 512], F32, tag=f"bank{i}", name=f"bank{i}")
                  for i in range(8)]

            # ---- persistent tiles (DMAs deferred past y4's first tile
            # so cam4's first energy matmul starts asap) ----
            down4 = persist.tile([P, S, 4, 256], F32)
            pooled = persist.tile([P, S, 28], F32)
            b4c = persist.tile([P, 4], F32)
            b2bc = persist.tile([P, 4], F32)
            fcw = persist.tile([P, 28, 2], F32)
            fcb = persist.tile([2, 1], F32)
            ident = persist.tile([P, 128], F32R)
            ones_row = persist.tile([1, 128], F32R)
            nc.vector.memset(_f32(ones_row[:]), 1.0)
            ones_col = persist.tile([P, 4], F32R)
            nc.vector.memset(_f32(ones_col[:]), 1.0)
            gb = {}
            for nm in ("g2", "g3", "g4"):
                gb[nm] = persist.tile([P, 1], F32, tag=f"gb_{nm}",
                                      name=f"gb_{nm}")

            def load_persist():
                nc.sync.dma_start(b4c[:], b4_d[:])
                nc.sync.dma_start(b2bc[:], b2b_d[:])
                nc.sync.dma_start(fcw[:],
                                  fcw_d[:].rearrange("(t p) o -> p t o", p=P))
                nc.sync.dma_start(fcb[:],
                                  fcb_d[:].rearrange("(a b) -> a b", b=1))
                nc.sync.dma_start(ident[:], ident_d[:])
                for nm, gd in (("g2", g2_d), ("g3", g3_d), ("g4", g4_d)):
                    nc.sync.dma_start(gb[nm][:], gd[:].to_broadcast((P, 1)))

            # ================= CAM4 + conv4 =================
            with tc.tile_pool(name="out4pool", bufs=1) as out4pool:
                out4 = out4pool.tile([P, S, 16, 256], F32R)
                x4 = out4pool.tile([P, S, 16, 256], F32R)
                with tc.tile_pool(name="cam4pool", bufs=1) as cam4pool:
                    E4 = cam4pool.tile([P, 16, 2048], F32R)
                    for s in range(S):
                        _mark(nc, f"cam4_s{s}")
                        y4 = cam4pool.tile([P, 2, 2048], F32R, tag="y4")
                        for k in range(2):
                            nc.sync.dma_start(
                                y4[:, k],
                                y4_d[s, k * P:(k + 1) * P, :])
                        if s == 0:
                            for s2 in range(S):
                                nc.sync.dma_start(
                                    x4[:, s2],
                                    x4_d[s2].rearrange("(k p) n -> p k n",
                                                       p=P))
                            load_persist()
                        def pool_sums(s=s, y4=y4):
                            # c4 spatial sums for pooled via tiny PE
                            # ones-matmuls over y4 (x^T) -- placed after
                            # energy so they fill the bias-chain PE bubble
                            pool_ps = PS[7][:, 0:64]
                            for mi in range(16):
                                for k2 in range(2):
                                    nc.tensor.matmul(
                                        pool_ps[:, mi * 4:(mi + 1) * 4],
                                        y4[:, k2, mi * P:(mi + 1) * P],
                                        ones_col[:],
                                        start=(k2 == 0), stop=(k2 == 1))
                            nc.vector.tensor_copy(pooled[:, s, 12:28],
                                                  pool_ps[:, 0:64:4])

                        def write4(m, c0, cn, ps, gzc, s=s):
                            nc.vector.scalar_tensor_tensor(
                                out4[:, s, m, :], ps, gzc,
                                _f32(x4[:, s, m, :]),
                                op0=OP.mult, op1=OP.add)

                        _cam_stage(
                            nc, name="c4", s=s, y_tile=y4,
                            x_view=lambda j, s=s: x4[:, s, j, :], E=E4,
                            PS=PS, gamma_b=gb["g4"], out_write=write4,
                            small=scratch, C=2048, N=256,
                            att_rhs_chunks=[(0, 256)],
                            ident=ident, ones_row=ones_row,
                            post_energy=pool_sums, dbg=dbg)

                # conv4: pam4 = w4 @ out4, down4 = w4 @ x4 (1x1 convs)
                # k-outer: all 8 output psums live; weights stream in 4
                # [P,4,512] chunks (8KB/partition DMAs, double-buffered)
                _mark(nc, "conv4")
                with tc.tile_pool(name="conv4pool", bufs=1) as conv4pool:
                    vpam = [PS[o][:].rearrange("p (s n) -> p s n", s=S)
                            for o in range(4)]
                    vdown = [PS[4 + o][:].rearrange("p (s n) -> p s n", s=S)
                             for o in range(4)]
                    # per-k weight DMAs: dependencies trickle in so matmul
                    # dispatches spread out (burst-dispatch after an idle
                    # costs LOW-pstate on every queued matmul)
                    for k in range(16):
                        wc = conv4pool.tile([P, 512], F32R, tag="w4k",
                                            bufs=4, name="wc4")
                        nc.sync.dma_start(
                            wc[:], w4_d[k * P:(k + 1) * P, :])
                        for o in range(4):
                            nc.tensor.matmul(
                                vpam[o], wc[:, o * 128:(o + 1) * 128],
                                out4[:, :, k, :],
                                start=(k == 0), stop=(k == 15))
                            nc.tensor.matmul(
                                vdown[o], wc[:, o * 128:(o + 1) * 128],
                                x4[:, :, k, :],
                                start=(k == 0), stop=(k == 15))
                    # down4 epilogues first: banks 4-7 free earliest and
                    # cam3's energy rotation starts there
                    for o in range(4):
                        for s in range(S):
                            # down4 = conv + b4 (kept for cam3/cam2 adds)
                            nc.scalar.activation(
                                down4[:, s, o, :], vdown[o][:, s, :],
                                AF.Identity, bias=b4c[:, o:o + 1], scale=1.0)
                    for o in range(4):
                        for s in range(S):
                            # cam4 = (pam4 + b4) + down4; only pooled sum kept
                            cam4_s = scratch.tile([P, 256], F32, tag="cam_scr",
                                                  name="cam4_s")
                            nc.vector.scalar_tensor_tensor(
                                cam4_s[:], vpam[o][:, s, :], b4c[:, o:o + 1],
                                down4[:, s, o, :], op0=OP.add, op1=OP.add,
                                accum_out=pooled[:, s, 8 + o:9 + o])

            if debug:
                for s2 in range(S):
                    for k in range(16):
                        nc.sync.dma_start(dbg["out4"][:, s2, k, :],
                                          out4[:, s2, k, :])
                    for o in range(4):
                        nc.sync.dma_start(dbg["down4"][:, s2, o, :],
                                          down4[:, s2, o, :])

            # ================= CAM3 + conv3 =================
            with tc.tile_pool(name="out3pool", bufs=1) as out3pool, \
                 tc.tile_pool(name="conv3pool", bufs=1) as conv3pool:
                out3p = out3pool.tile([P, S, 8, 34, 34], BF16)
                # zero only the 1-px padding ring (interior fully overwritten
                # by the att epilogue); Pool engine is idle
                nc.gpsimd.memset(out3p[:, :, :, 0:34:33, :], 0.0)
                nc.gpsimd.memset(out3p[:, :, :, 1:33, 0:34:33], 0.0)
                with tc.tile_pool(name="cam3pool", bufs=1) as cam3pool:
                    for s in range(S):
                        _mark(nc, f"cam3_s{s}")
                        E3 = cam3pool.tile([P, 8, 1024], F32R, tag="E3",
                                           bufs=1, name="E3")
                        y3 = cam3pool.tile([P, 8, 1024], F32R, tag="y3",
                                           bufs=1, name="y3")
                        for k in range(8):
                            nc.sync.dma_start(
                                y3[:, k], y3_d[s, k * P:(k + 1) * P, :])
                        x3 = cam3pool.tile([P, 8, 1024], F32R, tag="x3",
                                           bufs=2, name="x3")
                        for k in range(8):
                            nc.sync.dma_start(
                                x3[:, k], x3_d[s, k * P:(k + 1) * P, :])

                        def write3(m, c0, cn, ps, gzc, s=s, x3=x3):
                            r0 = c0 // 32          # spatial row offset
                            nr = cn // 32
                            nc.vector.scalar_tensor_tensor(
                                out3p[:, s, m, 1 + r0:1 + r0 + nr, 1:33],
                                ps.rearrange("p (a b) -> p a b", a=nr),
                                gzc,
                                _f32(x3[:, m, c0:c0 + cn]).rearrange(
                                    "p (a b) -> p a b", a=nr),
                                op0=OP.mult, op1=OP.add)

                        _cam_stage(
                            nc, name="c3", s=s, y_tile=y3,
                            x_view=lambda j, x3=x3: x3[:, j, :], E=E3,
                            PS=PS, gamma_b=gb["g3"], out_write=write3,
                            small=scratch, C=1024, N=1024,
                            att_rhs_chunks=[(0, 512), (512, 512)],
                            ident=ident, ones_row=ones_row,
                            ebanks=[4, 5, 6, 7, 0, 1, 2, 3], dbg=dbg)

                # conv3: 3x3 stride2 on out3p (padded) -> pam3; cam3 = pam3+down4
                _mark(nc, "conv3")
                vpam3 = [PS[o][:].rearrange("p (s a b) -> p s a b", s=S, a=16)
                         for o in range(4)]
                for t9 in range(9):
                    ky, kx = t9 // 3, t9 % 3
                    wt = conv3pool.tile([P, 8, 512], BF16, tag="w3tap",
                                        bufs=2, name="w3tap")
                    nc.sync.dma_start(wt[:], w3_d[t9].rearrange(
                        "(k p) o -> p k o", p=P))
                    for o in range(4):
                        for k in range(8):
                            rhs = out3p[:, :, k, ky:ky + 32:2,
                                        kx:kx + 32:2]
                            nc.tensor.matmul(
                                vpam3[o],
                                wt[:, k, o * 128:(o + 1) * 128],
                                rhs,
                                start=(t9 == 0 and k == 0),
                                stop=(t9 == 8 and k == 7))
                for s in range(S):
                    for o in range(4):
                        cam3_s = scratch.tile([P, 256], F32, tag="cam_scr",
                                              name="cam3_s")
                        nc.vector.scalar_tensor_tensor(
                            cam3_s[:],
                            vpam3[o][:, s].rearrange("p a b -> p (a b)"),
                            0.0,
                            down4[:, s, o, :], op0=OP.add, op1=OP.add,
                            accum_out=pooled[:, s, 4 + o:5 + o])

            if debug:
                for s2 in range(S):
                    for k in range(8):
                        nc.sync.dma_start(dbg["out3p"][:, s2, k], out3p[:, s2, k])

            # ================= CAM2 + conv2a + conv2b (per sample) ==========
            with tc.tile_pool(name="cam2outer", bufs=1) as cam2outer, \
                 tc.tile_pool(name="c2apool", bufs=1) as c2apool, \
                 tc.tile_pool(name="c2bpool", bufs=1) as c2bpool, \
                 tc.tile_pool(name="cam2pool", bufs=1) as cam2pool:
                out2p = cam2outer.tile([P, 4, 66, 66], BF16)
                a2p = cam2outer.tile([P, 4, 34, 34], BF16)
                E2 = cam2outer.tile([P, 4, 512], F32R)
                PT2 = cam2outer.tile([P, 4, 512], F32R)
                nc.gpsimd.memset(out2p[:, :, 0:66:65, :], 0.0)
                nc.gpsimd.memset(out2p[:, :, 1:65, 0:66:65], 0.0)
                nc.gpsimd.memset(a2p[:, :, 0:34:33, :], 0.0)
                nc.gpsimd.memset(a2p[:, :, 1:33, 0:34:33], 0.0)
                # w2b resident in bf16 for BOTH samples (36KB, loaded once)
                w2bres = c2bpool.tile([P, 9, 4, 512], BF16)
                for t9 in range(9):
                    nc.sync.dma_start(w2bres[:, t9], w2b_d[t9].rearrange(
                        "(k p) o -> p k o", p=P))
                for s in range(S):
                    _mark(nc, f"cam2_s{s}")
                    # CAM2's row-min spread (up to ~350) exceeds any single
                    # fp32 exp window, so it uses the exact per-row
                    # stabilizer and pays 16 cheap PE transposes for p^T.
                    mins = scratch.tile([P, 4], F32, tag="mins2",
                                        name="mins2")
                    for k in range(32):
                        yk = cam2pool.tile([P, 512], F32R, tag="ystream",
                                           bufs=8, name="yk")
                        nc.sync.dma_start(yk[:],
                                          y2_d[s, k * P:(k + 1) * P, :])
                        for m in range(4):
                            nc.tensor.matmul(
                                PS[m][:], yk[:, m * P:(m + 1) * P],
                                yk[:], start=(k == 0), stop=(k == 31))
                    z2 = scratch.tile([P, 4], F32, tag="z_c2", name="z2")
                    for m in range(4):
                        nc.vector.tensor_reduce(
                            mins[:, m:m + 1], PS[m][:],
                            axis=AX.X, op=OP.min)
                        # p = exp(m_c - E) straight from PSUM (per-row bias
                        # is known pre-copy, so no raw-E copy is needed)
                        nc.scalar.activation(
                            E2[:, m, :], PS[m][:], AF.Exp,
                            bias=mins[:, m:m + 1], scale=-1.0,
                            accum_out=z2[:, m:m + 1])
                    rz2 = scratch.tile([P, 4], F32, tag="rz_c2",
                                       name="rz2")
                    nc.vector.reciprocal(rz2[:], z2[:])
                    gz2 = scratch.tile([P, 4], F32, tag="gz_c2",
                                       name="gz2")
                    nc.vector.tensor_scalar(gz2[:], rz2[:], gb["g2"][:, 0:1],
                                            None, op0=OP.mult)
                    if dbg is not None:
                        nc.sync.dma_start(dbg[f"z_c2_{s}"][:], z2[:])
                        nc.sync.dma_start(dbg[f"K0_c2_{s}"][:],
                                          _f32(E2[:, 0, :]))

                    # PT2 = p^T via PE transposes of 128x128 blocks
                    for i in range(4):
                        for j in range(4):
                            tp = PS[4 + ((i * 4 + j) % 2)][:, 0:128]
                            nc.tensor.transpose(
                                tp.bitcast(F32R),
                                E2[:, i, j * P:(j + 1) * P], ident[:])
                            if (i * 4 + j) % 2:
                                nc.vector.tensor_copy(
                                    PT2[:, j, i * P:(i + 1) * P], tp)
                            else:
                                nc.scalar.activation(
                                    PT2[:, j, i * P:(i + 1) * P],
                                    tp, AF.Identity)

                    # att @ x2 in 8 column chunks of 512 (8 spatial rows)
                    for c in range(8):
                        xc = cam2pool.tile([P, 4, 512], F32R, tag="x2c",
                                           bufs=4, name="x2c")
                        for j in range(4):
                            nc.sync.dma_start(
                                xc[:, j],
                                x2_d[s, j * P:(j + 1) * P,
                                     c * 512:(c + 1) * 512])
                        base = (c % 2) * 4
                        for m in range(4):
                            ps = PS[base + m]
                            for j in range(4):
                                nc.tensor.matmul(
                                    ps[:], PT2[:, j, m * P:(m + 1) * P],
                                    xc[:, j, :],
                                    start=(j == 0), stop=(j == 3))
                            nc.vector.scalar_tensor_tensor(
                                out2p[:, m, 1 + 8 * c:9 + 8 * c, 1:65],
                                ps[:].rearrange("p (a b) -> p a b", a=8),
                                gz2[:, m:m + 1],
                                _f32(xc[:, m, :]).rearrange(
                                    "p (a b) -> p a b", a=8),
                                op0=OP.mult, op1=OP.add)

                    # conv2a: 64x64 -> 32x32 (no bias), output padded a2p
                    _mark(nc, f"conv2a_s{s}")
                    vpa = [PS[i][:].rearrange("p (a b) -> p a b", a=32)
                           for i in range(8)]
                    for t9 in range(9):
                        ky, kx = t9 // 3, t9 % 3
                        wt = c2apool.tile([P, 4, 512], BF16, tag="w2atap",
                                          bufs=2, name="w2atap")
                        nc.sync.dma_start(wt[:], w2a_d[t9].rearrange(
                            "(k p) o -> p k o", p=P))
                        for o in range(4):
                            for h in range(2):
                                for k in range(4):
                                    rhs = out2p[:, k, ky:ky + 64:2,
                                                kx + 32 * h:
                                                kx + 32 * h + 32:2]
                                    nc.tensor.matmul(
                                        vpa[o * 2 + h],
                                        wt[:, k, o * 128:(o + 1) * 128],
                                        rhs,
                                        start=(t9 == 0 and k == 0),
                                        stop=(t9 == 8 and k == 3))
                    for o in range(4):
                        for h in range(2):
                            if (o * 2 + h) % 2:
                                nc.vector.tensor_copy(
                                    a2p[:, o, 1:33,
                                        1 + 16 * h:17 + 16 * h],
                                    vpa[o * 2 + h])
                            else:
                                nc.scalar.activation(
                                    a2p[:, o, 1:33,
                                        1 + 16 * h:17 + 16 * h],
                                    vpa[o * 2 + h], AF.Identity)

                    # conv2b: 32x32 -> 16x16 (+b2b), cam2 = pam2 + b2b + down4
                    _mark(nc, f"conv2b_s{s}")
                    vpb = [PS[o][:, 0:256] for o in range(4)]
                    for t9 in range(9):
                        ky, kx = t9 // 3, t9 % 3
                        for o in range(4):
                            for k in range(4):
                                rhs = a2p[:, k, ky:ky + 32:2,
                                          kx:kx + 32:2]
                                nc.tensor.matmul(
                                    vpb[o],
                                    w2bres[:, t9, k, o * 128:(o + 1) * 128],
                                    rhs,
                                    start=(t9 == 0 and k == 0),
                                    stop=(t9 == 8 and k == 3))
                    for o in range(4):
                        cam2_s = scratch.tile([P, 256], F32, tag="cam_scr",
                                              name="cam2_s")
                        nc.vector.scalar_tensor_tensor(
                            cam2_s[:], vpb[o], b2bc[:, o:o + 1],
                            down4[:, s, o, :], op0=OP.add, op1=OP.add,
                            accum_out=pooled[:, s, o:o + 1])

            # ================= FC =================
            _mark(nc, "fc")
            if debug:
                nc.sync.dma_start(dbg["pooled"][:], pooled[:])
            for s in range(S):
                pfc = PS[6 + s][0:2, 0:1]
                for t in range(28):
                    nc.tensor.matmul(
                        pfc, fcw[:, t, :], pooled[:, s, t:t + 1],
                        start=(t == 0), stop=(t == 27))
                fc_o = scratch.tile([2, 1], F32, tag="fc_o", name="fc_o")
                nc.vector.tensor_tensor(fc_o[:], pfc, fcb[:], op=OP.add)
                nc.sync.dma_start(
                    out_d[s].rearrange("(a b) -> a b", b=1), fc_o[:])

    return nc


def prepare_in_maps(c2, c3, c4, w4, b4, w3, w2a, w2b, b2b, g2, g3, g4,
                    fc_w, fc_b):
    B = c2.shape[0]
    f32 = np.float32
    c2f = np.ascontiguousarray(c2, dtype=f32).reshape(B, 512, 4096)
    c3f = np.ascontiguousarray(c3, dtype=f32).reshape(B, 1024, 1024)
    c4f = np.ascontiguousarray(c4, dtype=f32).reshape(B, 2048, 256)
    y2 = np.ascontiguousarray(c2f.transpose(0, 2, 1))
    y3 = np.ascontiguousarray(c3f.transpose(0, 2, 1))
    y4 = np.ascontiguousarray(c4f.transpose(0, 2, 1))

    w4t = np.ascontiguousarray(w4[:, :, 0, 0].T, dtype=f32)           # [2048,512]
    import ml_dtypes as _mld
    w3t = np.ascontiguousarray(
        w3.transpose(2, 3, 1, 0).reshape(9, 1024, 512)).astype(
            _mld.bfloat16)
    import ml_dtypes
    bf16 = ml_dtypes.bfloat16
    w2at = np.ascontiguousarray(
        w2a.transpose(2, 3, 1, 0).reshape(9, 512, 512)).astype(bf16)
    w2bt = np.ascontiguousarray(
        w2b.transpose(2, 3, 1, 0).reshape(9, 512, 512)).astype(bf16)
    b4c = np.ascontiguousarray(np.asarray(b4, f32).reshape(4, 128).T)
    b2bc = np.ascontiguousarray(np.asarray(b2b, f32).reshape(4, 128).T)
    fcwt = np.ascontiguousarray(np.asarray(fc_w, f32).T / 256.0)      # [3584,2]

    in_maps = []
    for core in range(NCORES):
        s0 = core * SAMPLES_PER_CORE
        sl = slice(s0, s0 + SAMPLES_PER_CORE)
        in_maps.append({
            "x2": np.ascontiguousarray(c2f[sl]),
            "y2": np.ascontiguousarray(y2[sl]),
            "x3": np.ascontiguousarray(c3f[sl]),
            "y3": np.ascontiguousarray(y3[sl]),
            "x4": np.ascontiguousarray(c4f[sl]),
            "y4": np.ascontiguousarray(y4[sl]),
            "w4t": w4t, "w3t": w3t, "w2at": w2at, "w2bt": w2bt,
            "b4c": b4c, "b2bc": b2bc, "fcwt": fcwt,
            "fcb": np.asarray(fc_b, f32),
            "g2": np.asarray(g2, f32), "g3": np.asarray(g3, f32),
            "g4": np.asarray(g4, f32),
            "ident": np.eye(128, dtype=f32),
        })
    return in_maps


def kernel(**inputs):
    nc = build_program()
    in_maps = prepare_in_maps(**inputs)
    res = run_bass_kernel_spmd(nc, in_maps, core_ids=list(range(NCORES)))
    out = np.concatenate([r["out"] for r in res.results], axis=0)
    return out.astype(np.float32)
